# revision 55
# baseline (speedup 1.0000x reference)
"""PrRoIPool (Precise RoI Pooling) Trainium2 Bass kernel.

Problem: features [2, 256, 100, 100] f32, rois [256, 5] f32 ->
out [256, 256, 7, 7] f32 where
  out[n,c,p,q] = (1/area) * sum_{h,w} F[bi,c,h,w] * wy[n,p,h] * wx[n,q,w]
with wy/wx the exact integrals of the bilinear-interp hat functions over
each pooling bin (separable).

Strategy (8 NeuronCores, SPMD):
  - Host: compute hat-integral weights wy [N,7,H], wx [N,7,W] (tiny:
    ~0.004%% of total FLOPs), fold 1/bin_h into wy and 1/bin_w into wx.
  - Shard ROIs by batch image: cores 0-3 take batch-0 ROIs, cores 4-7
    batch-1 (S slots per core, zero-padded). Each core holds the full
    feature image of its batch, pre-transposed to [W, C, H] fp16.
  - Stage A (per channel c): T1[h, (s,q)] = F[c].T @ wx  -- PE matmul,
    stationary [w=100, h=100], moving [w=100, S*7], fp32 PSUM, then
    cast-copy PSUM->SBUF fp16 (Vector/Scalar engines, load-balanced).
  - Stage B (per 4-ROI group, per 64-channel quarter j):
    out[p, (c',q)] = wy_s.T @ T1[:, s, 64j:64j+64, :]  -- 4 ROIs packed
    at PSUM partition offsets 0/32/64/96 via matmul tile_position; the 4
    matmuls run concurrently in distinct array col-groups.
  - Output DMA ships only the 7 valid rows of each 32-row slot block.
  - Host reassembles [N, C, 7, 7].
"""

import sys

if "/opt/trn_rl_repo" not in sys.path:
    sys.path.insert(0, "/opt/trn_rl_repo")

import numpy as np

POOLED = 7
SPATIAL_SCALE = 0.0625
B, C, H, W = 2, 256, 100, 100
N_CORES = 8
CORES_PER_BATCH = 4
S_CAP = 36  # max ROI slots per core (stage-A PSUM slot is half a bank)

_prog_cache = {}

# feature-chunk schedule: small chunks first for a fast pipeline start.
# Chunks are channel-contiguous; the host packs the feature image into a
# flat DRAM buffer as consecutive [W, chunk, H] blocks so every chunk DMA
# reads fully contiguous DRAM (strided reads of [W, C, H] run at ~80 GB/s
# and starve stage A).
_CHUNKS = []
for _q in ([2, 2, 4, 8, 16, 16, 16], [16, 16, 16, 16],
           [16, 16, 16, 16], [16, 16, 16, 16]):
    _CHUNKS.extend(_q)
_STARTS = np.cumsum([0] + _CHUNKS).tolist()


def _hat_cdf(u):
    return np.where(
        u <= 0.0,
        0.5 * np.clip(u + 1.0, 0.0, 1.0) ** 2,
        1.0 - 0.5 * np.clip(1.0 - u, 0.0, 1.0) ** 2,
    )


def _bin_weights(lo, hi, size):
    # [N, P] bounds -> [N, P, size] integral of hat centered at each index
    idx = np.arange(size, dtype=lo.dtype)
    return _hat_cdf(hi[..., None] - idx) - _hat_cdf(lo[..., None] - idx)


def _host_weights(rois):
    """Per-ROI separable weights with 1/area folded in. float32."""
    r = rois.astype(np.float64)
    x1 = r[:, 1] * SPATIAL_SCALE
    y1 = r[:, 2] * SPATIAL_SCALE
    x2 = r[:, 3] * SPATIAL_SCALE
    y2 = r[:, 4] * SPATIAL_SCALE
    bw = (x2 - x1) / POOLED
    bh = (y2 - y1) / POOLED
    ph = np.arange(POOLED, dtype=np.float64)
    ylo = y1[:, None] + ph * bh[:, None]
    yhi = ylo + bh[:, None]
    xlo = x1[:, None] + ph * bw[:, None]
    xhi = xlo + bw[:, None]
    wy = _bin_weights(ylo, yhi, H)  # [N, 7, H]
    wx = _bin_weights(xlo, xhi, W)  # [N, 7, W]
    # reference: out = einsum / max(area,1e-12) where area = bw*bh, zeroed
    # if area <= 0. Fold 1/bh into wy, 1/bw into wx (area > 0 case).
    ok = (bw * bh) > 0.0
    inv_bh = np.where(ok, 1.0 / np.maximum(bh, 1e-12), 0.0)
    inv_bw = np.where(ok, 1.0 / np.maximum(bw, 1e-12), 0.0)
    wy = wy * inv_bh[:, None, None]
    wx = wx * inv_bw[:, None, None]
    return wy.astype(np.float32), wx.astype(np.float32)


def _build_program(S):
    """Bass/Tile SPMD program for S ROI slots per core. Cached per S."""
    from contextlib import ExitStack

    from concourse import bacc, mybir
    import concourse.tile as tile

    f16 = mybir.dt.float16
    f32 = mybir.dt.float32
    SQ = S * POOLED
    assert SQ <= 256  # stage-A psum: 2 channel slots of 256 f32 = 1 bank
    NG = -(-S // 4)  # 4-slot stage-B groups

    nc = bacc.Bacc("TRN2", target_bir_lowering=False, debug=False,
                   num_devices=N_CORES)
    fwt = nc.dram_tensor("fwt", [W * C * H], f16, kind="ExternalInput")
    wxt = nc.dram_tensor("wxt", [W, SQ], f16, kind="ExternalInput")
    # wyt is padded to 32 cols per slot (25 zero) so each stage-B matmul
    # writes a full 32-partition PSUM block (cost is N-driven, M is free).
    wyt = nc.dram_tensor("wyt", [H, S * 32], f16, kind="ExternalInput")
    # staged output, fp16: per (group, channel-half) rows 0-103 of the
    # 4x32-row slot blocks (rows 32i..32i+6 valid); one DMA each keeps
    # the descriptor-gen cost low, and the 104-row block size keeps
    # every (g, hf) DRAM block 2KB-aligned (103 rows shatters the DMA
    # into ~350B packets and the drain takes ~60us)
    out = nc.dram_tensor("out", [NG, 2, 104, 2, 448], f16,
                         kind="ExternalOutput")

    chunks, starts = _CHUNKS, _STARTS

    # strict V/S alternation: the two copy engines run concurrently on
    # different PSUM banks; any consecutive same-engine pair serializes
    # on that engine's DRAIN while the other idles
    state = {"n_copy": 0}

    with tile.TileContext(nc) as tc, ExitStack() as ctx:
        sb = ctx.enter_context(tc.tile_pool(name="sb", bufs=1))
        fw_pool = ctx.enter_context(tc.tile_pool(name="fw", bufs=7))
        pa_pool = ctx.enter_context(tc.tile_pool(name="pa", bufs=3,
                                                 space="PSUM"))
        pb_pool = ctx.enter_context(tc.tile_pool(name="pb", bufs=2,
                                                 space="PSUM"))
        stg_pool = ctx.enter_context(tc.tile_pool(name="stg", bufs=NG + 2))

        def bal_copy(dst, src, fd):
            if state["n_copy"] % 2 == 0:
                nc.vector.tensor_copy(dst, src)
            else:
                nc.scalar.copy(dst, src)
            state["n_copy"] += 1

        wx_t = sb.tile([W, SQ], f16, tag="wx")
        nc.sync.dma_start(out=wx_t[:], in_=wxt[:])
        wy_t = sb.tile([H, S * 32], f16, tag="wy")

        # PE warmup: dense matmuls while the first feature DMAs land, to
        # flip the HAM clock gate to 8/8 (PE runs at 1.2 GHz until it sees
        # a ~3.4us window of sustained activity; everything after runs 2x)
        warm = sb.tile([128, 640], f16, tag="warm")
        nc.gpsimd.memset(warm[:], 0.0)
        # warmup matmuls use a pb-pool bank; quarter 0 has no stage-B
        # work, so this never blocks a copy
        wps = pb_pool.tile([128, 512], f32, tag="pb")
        for _ in range(10):
            nc.tensor.matmul(wps[:, 0:512], lhsT=warm[:, 0:128],
                             rhs=warm[:, 128:640])
        # per-quarter T1 tiles so stage-B reads of quarter j never create
        # false WAR deps against stage-A writes of quarter j+1
        t1 = [sb.tile([H, S, 64, POOLED], f16, tag=f"t1_{j}",
                      name=f"t1_{j}") for j in range(4)]

        chunk_idx = 0
        cur = None
        pa = None
        stgs = {}

        def keepalive(n=512):
            # real matmul: only MAC activity feeds the HAM activity
            # monitor (LDWEIGHTS/transpose don't count)
            ka = pb_pool.tile([128, 512], f32, tag="pb")
            nc.tensor.matmul(ka[:, 0:n], lhsT=warm[:, 0:128],
                             rhs=warm[:, 128:128 + n])

        def keepalive_pa():
            # tail-only keepalive: the pa pool is idle once stage A ends,
            # so these never steal a pb bank -- keepalives from pb during
            # the tail pin pb allocations to one alternating bank and
            # serialize each emit behind the previous emit's copy
            # (~950ns/emit instead of the copy-paced ~600ns)
            ka = pa_pool.tile([H, 4, 256], f32, tag="pa")
            nc.tensor.matmul(ka[:, 0, 0:256], lhsT=warm[:, 0:100],
                             rhs=warm[:, 128:384])

        def emit_b_group(j, g):
            # stage-B for quarter j, 4-ROI group g at PSUM partition
            # offsets 0/32/64/96 (concurrent col-group matmuls)
            hf, jj = divmod(j, 2)
            if jj == 0:
                stgs[g] = stg_pool.tile([128, 2, 448], f16, tag="stg",
                                        name=f"stg_{hf}_{g}")
            stg = stgs[g]
            g0 = 4 * g
            rois_g = list(range(g0, min(g0 + 4, S)))
            hi = 32 * len(rois_g)
            c0, c1 = 0, 64
            pb = pb_pool.tile([128, 512], f32, tag="pb")
            for i, s in enumerate(rois_g):
                nc.tensor.matmul(
                    pb[32 * i:32 * i + 32, 0:448],
                    lhsT=wy_t[:, s * 32:(s + 1) * 32],
                    rhs=t1[j][:, s, c0:c1, :],
                    tile_position=(0, 32 * i),
                )
            bal_copy(stg[0:hi, jj, :], pb[0:hi, 0:448], 448)
            if jj == 1:
                lo = min(hi + 8, 104)
                # final-quarter outputs alternate trigger engines: the
                # tail is drain-bound on a single DMA ring, and sync is
                # idle once input descriptors are done
                eng = nc.sync if (j == 3 and g % 2 == 1) else nc.gpsimd
                eng.dma_start(out=out[g, hf, 0:lo], in_=stg[0:lo])

        for j in range(4):
            # ---- Stage A quarter: T1_j[h, s, c', q] = F[c].T @ wx ----
            # with the previous quarter's stage-B groups interleaved every
            # 8 channels to fill the copy-paced gaps on the PE
            pending_b = list(range(NG)) if j > 0 else []
            for ci, c in enumerate(range(64 * j, 64 * (j + 1))):
                if c == starts[chunk_idx]:
                    sz = chunks[chunk_idx]
                    cur = fw_pool.tile([W, 16, H], f16, tag="fw")
                    src = fwt[c * W * H:(c + sz) * W * H].rearrange(
                        "(w cc h) -> w cc h", w=W, h=H)
                    nc.sync.dma_start(out=cur[:, 0:sz, :], in_=src)
                    chunk_idx += 1
                    coff = c
                    if c == 16:
                        # wy is first needed by stage-B of quarter 0,
                        # i.e. early in quarter 1; don't let it delay
                        # the first feature chunks
                        nc.sync.dma_start(out=wy_t[:], in_=wyt[:])
                if c % 4 == 0:
                    pa = pa_pool.tile([H, 4, 256], f32, tag="pa")
                nc.tensor.matmul(
                    pa[:, c % 4, 0:SQ],
                    lhsT=cur[:, c - coff, :],
                    rhs=wx_t[:],
                )
                if c % 4 == 3:
                    src = pa[:, :, 0:SQ].rearrange("h c (s q) -> h s c q",
                                                   q=POOLED)
                    dst = t1[j][:, :, (c - 3) % 64:(c - 3) % 64 + 4, :]
                    bal_copy(dst, src, 4 * SQ)
                if ci % 7 == 4 and pending_b:
                    emit_b_group(j - 1, pending_b.pop(0))
                elif j == 0 and (ci % 8 == 4 or (ci < 32 and ci % 4 == 1)):
                    # no stage-B work yet: keepalive matmul to hold the
                    # HAM clock gate at 8/8. Denser in the first half of
                    # quarter 0: the DMA-paced ramp leaves low-activity
                    # windows there, and a single core hitting a ~3.4us
                    # idle window re-throttles and drags the whole
                    # max-over-cores exec time
                    keepalive()
            for g in pending_b:
                emit_b_group(j - 1, g)
            # bridge keepalive over the quarter boundary (the next quarter's
            # first B group waits on this quarter's last copy; a PE hole
            # here trips the HAM re-throttle)
            keepalive()
        # extra bridge before the tail: B(3) group 0 waits the last A-copy
        for _ in range(2):
            keepalive_pa()
        # last quarter's stage B (dense tail) with keepalives
        for g in range(NG):
            emit_b_group(3, g)
            keepalive_pa()

    nc.compile()
    return nc


def _plan_shards(bi, n_rois):
    """Assign ROI indices to (wave, core, slot). Returns S and a list of
    per-wave assignment arrays of shape [N_CORES, S] (-1 = padding)."""
    groups = [np.where(bi == b)[0] for b in range(B)]
    need = max((len(g) + CORES_PER_BATCH - 1) // CORES_PER_BATCH
               for g in groups)
    need = max(need, 1)
    S = min(need, S_CAP)
    per_wave_cap = S * CORES_PER_BATCH
    n_waves = max(-(-len(g) // per_wave_cap) for g in groups)
    waves = []
    for wv in range(n_waves):
        asg = np.full((N_CORES, S), -1, dtype=np.int64)
        for b in range(B):
            g = groups[b][wv * per_wave_cap:(wv + 1) * per_wave_cap]
            for k in range(CORES_PER_BATCH):
                chunk = g[k * S:(k + 1) * S]
                asg[b * CORES_PER_BATCH + k, :len(chunk)] = chunk
        waves.append(asg)
    return S, waves


def kernel(features, rois, _trace=False):
    from concourse.bass_utils import run_bass_kernel_spmd

    features = np.asarray(features, dtype=np.float32)
    rois = np.asarray(rois, dtype=np.float32)
    n_rois = rois.shape[0]
    bi = np.rint(rois[:, 0]).astype(np.int64)
    bi = np.where((bi >= 0) & (bi < B), bi, -1)

    wy, wx = _host_weights(rois)  # [N, 7, H] / [N, 7, W], 1/area folded
    S, waves = _plan_shards(bi, n_rois)

    if S not in _prog_cache:
        _prog_cache[S] = _build_program(S)
    nc = _prog_cache[S]

    # Features per batch as a flat buffer of chunk-contiguous [W, sz, H]
    # blocks (fast contiguous DRAM reads). Shared across the 4 cores of
    # each batch group.
    fwt = []
    for b in range(B):
        t = features[b].transpose(2, 0, 1).astype(np.float16)  # [W, C, H]
        fwt.append(np.concatenate(
            [t[:, c0:c1, :].ravel()
             for c0, c1 in zip(_STARTS[:-1], _STARTS[1:])]))

    out_full = np.zeros((n_rois, C, POOLED, POOLED), dtype=np.float32)
    exec_ns = None
    for asg in waves:
        in_maps = []
        for k in range(N_CORES):
            wxt = np.zeros((W, S * POOLED), dtype=np.float16)
            wyt = np.zeros((H, S * 32), dtype=np.float16)
            for s in range(S):
                r = asg[k, s]
                if r < 0:
                    continue
                # w[n, q, w-axis] -> [w-axis, s*7+q]
                wxt[:, s * POOLED:(s + 1) * POOLED] = \
                    wx[r].T.astype(np.float16)
                wyt[:, s * 32:s * 32 + POOLED] = \
                    wy[r].T.astype(np.float16)
            in_maps.append({
                "fwt": fwt[k // CORES_PER_BATCH],
                "wxt": wxt,
                "wyt": wyt,
            })
        res = run_bass_kernel_spmd(nc, in_maps, list(range(N_CORES)),
                                   trace=_trace)
        if res.exec_time_ns is not None:
            exec_ns = max(exec_ns or 0, res.exec_time_ns)
        for k in range(N_CORES):
            arr = res.results[k]["out"]  # [NG, 2, 104, 2, 448] f16
            for s in range(S):
                r = asg[k, s]
                if r < 0:
                    continue
                g, i = divmod(s, 4)
                # [hf, p, jj, c', q] -> [hf, jj, c', p, q] -> [C, 7, 7]
                blk = (arr[g, :, 32 * i:32 * i + POOLED]
                       .reshape(2, POOLED, 2, 64, POOLED)
                       .transpose(0, 2, 3, 1, 4)
                       .reshape(C, POOLED, POOLED))
                out_full[r] = blk.astype(np.float32)

    if _trace:
        kernel.last_exec_time_ns = exec_ns
    return out_full


# revision 56
# speedup vs baseline: 1.0326x; 1.0326x over previous
"""PrRoIPool (Precise RoI Pooling) Trainium2 Bass kernel.

Problem: features [2, 256, 100, 100] f32, rois [256, 5] f32 ->
out [256, 256, 7, 7] f32 where
  out[n,c,p,q] = (1/area) * sum_{h,w} F[bi,c,h,w] * wy[n,p,h] * wx[n,q,w]
with wy/wx the exact integrals of the bilinear-interp hat functions over
each pooling bin (separable).

Strategy (8 NeuronCores, SPMD):
  - Host: compute hat-integral weights wy [N,7,H], wx [N,7,W] (tiny:
    ~0.004%% of total FLOPs), fold 1/bin_h into wy and 1/bin_w into wx.
  - Shard ROIs by batch image: cores 0-3 take batch-0 ROIs, cores 4-7
    batch-1 (S slots per core, zero-padded). Each core holds the full
    feature image of its batch, pre-transposed to [W, C, H] fp16.
  - Stage A (per channel c): T1[h, (s,q)] = F[c].T @ wx  -- PE matmul,
    stationary [w=100, h=100], moving [w=100, S*7], fp32 PSUM, then
    cast-copy PSUM->SBUF fp16 (Vector/Scalar engines, load-balanced).
  - Stage B (per 4-ROI group, per 64-channel quarter j):
    out[p, (c',q)] = wy_s.T @ T1[:, s, 64j:64j+64, :]  -- 4 ROIs packed
    at PSUM partition offsets 0/32/64/96 via matmul tile_position; the 4
    matmuls run concurrently in distinct array col-groups.
  - Output DMA ships only the 7 valid rows of each 32-row slot block.
  - Host reassembles [N, C, 7, 7].
"""

import sys

if "/opt/trn_rl_repo" not in sys.path:
    sys.path.insert(0, "/opt/trn_rl_repo")

import numpy as np

POOLED = 7
SPATIAL_SCALE = 0.0625
B, C, H, W = 2, 256, 100, 100
N_CORES = 8
CORES_PER_BATCH = 4
S_CAP = 36  # max ROI slots per core (stage-A PSUM slot is half a bank)

_prog_cache = {}

# feature-chunk schedule: small chunks first for a fast pipeline start.
# Chunks are channel-contiguous; the host packs the feature image into a
# flat DRAM buffer as consecutive [W, chunk, H] blocks so every chunk DMA
# reads fully contiguous DRAM (strided reads of [W, C, H] run at ~80 GB/s
# and starve stage A).
_CHUNKS = []
for _q in ([2, 2, 4, 8, 16, 16, 16], [16, 16, 16, 16],
           [16, 16, 16, 16], [16, 16, 16, 16]):
    _CHUNKS.extend(_q)
_STARTS = np.cumsum([0] + _CHUNKS).tolist()


def _hat_cdf(u):
    return np.where(
        u <= 0.0,
        0.5 * np.clip(u + 1.0, 0.0, 1.0) ** 2,
        1.0 - 0.5 * np.clip(1.0 - u, 0.0, 1.0) ** 2,
    )


def _bin_weights(lo, hi, size):
    # [N, P] bounds -> [N, P, size] integral of hat centered at each index
    idx = np.arange(size, dtype=lo.dtype)
    return _hat_cdf(hi[..., None] - idx) - _hat_cdf(lo[..., None] - idx)


def _host_weights(rois):
    """Per-ROI separable weights with 1/area folded in. float32."""
    r = rois.astype(np.float64)
    x1 = r[:, 1] * SPATIAL_SCALE
    y1 = r[:, 2] * SPATIAL_SCALE
    x2 = r[:, 3] * SPATIAL_SCALE
    y2 = r[:, 4] * SPATIAL_SCALE
    bw = (x2 - x1) / POOLED
    bh = (y2 - y1) / POOLED
    ph = np.arange(POOLED, dtype=np.float64)
    ylo = y1[:, None] + ph * bh[:, None]
    yhi = ylo + bh[:, None]
    xlo = x1[:, None] + ph * bw[:, None]
    xhi = xlo + bw[:, None]
    wy = _bin_weights(ylo, yhi, H)  # [N, 7, H]
    wx = _bin_weights(xlo, xhi, W)  # [N, 7, W]
    # reference: out = einsum / max(area,1e-12) where area = bw*bh, zeroed
    # if area <= 0. Fold 1/bh into wy, 1/bw into wx (area > 0 case).
    ok = (bw * bh) > 0.0
    inv_bh = np.where(ok, 1.0 / np.maximum(bh, 1e-12), 0.0)
    inv_bw = np.where(ok, 1.0 / np.maximum(bw, 1e-12), 0.0)
    wy = wy * inv_bh[:, None, None]
    wx = wx * inv_bw[:, None, None]
    return wy.astype(np.float32), wx.astype(np.float32)


def _build_program(S):
    """Bass/Tile SPMD program for S ROI slots per core. Cached per S."""
    from contextlib import ExitStack

    from concourse import bacc, mybir
    import concourse.tile as tile

    f16 = mybir.dt.float16
    f32 = mybir.dt.float32
    SQ = S * POOLED
    assert SQ <= 256  # stage-A psum: 2 channel slots of 256 f32 = 1 bank
    NG = -(-S // 4)  # 4-slot stage-B groups

    nc = bacc.Bacc("TRN2", target_bir_lowering=False, debug=False,
                   num_devices=N_CORES)
    fwt = nc.dram_tensor("fwt", [W * C * H], f16, kind="ExternalInput")
    wxt = nc.dram_tensor("wxt", [W, SQ], f16, kind="ExternalInput")
    # wyt is padded to 32 cols per slot (25 zero) so each stage-B matmul
    # writes a full 32-partition PSUM block (cost is N-driven, M is free).
    wyt = nc.dram_tensor("wyt", [H, S * 32], f16, kind="ExternalInput")
    # staged output, fp16: per (group, channel-half) rows 0-103 of the
    # 4x32-row slot blocks (rows 32i..32i+6 valid); one DMA each keeps
    # the descriptor-gen cost low, and the 104-row block size keeps
    # every (g, hf) DRAM block 2KB-aligned (103 rows shatters the DMA
    # into ~350B packets and the drain takes ~60us)
    out = nc.dram_tensor("out", [NG, 2, 104, 2, 448], f16,
                         kind="ExternalOutput")

    chunks, starts = _CHUNKS, _STARTS

    # strict V/S alternation: the two copy engines run concurrently on
    # different PSUM banks; any consecutive same-engine pair serializes
    # on that engine's DRAIN while the other idles
    state = {"n_copy": 0}

    with tile.TileContext(nc) as tc, ExitStack() as ctx:
        sb = ctx.enter_context(tc.tile_pool(name="sb", bufs=1))
        fw_pool = ctx.enter_context(tc.tile_pool(name="fw", bufs=7))
        pa_pool = ctx.enter_context(tc.tile_pool(name="pa", bufs=3,
                                                 space="PSUM"))
        pb_pool = ctx.enter_context(tc.tile_pool(name="pb", bufs=2,
                                                 space="PSUM"))
        stg_pool = ctx.enter_context(tc.tile_pool(name="stg", bufs=NG + 2))

        def bal_copy(dst, src, fd):
            if state["n_copy"] % 2 == 0:
                nc.vector.tensor_copy(dst, src)
            else:
                nc.scalar.copy(dst, src)
            state["n_copy"] += 1

        wx_t = sb.tile([W, SQ], f16, tag="wx")
        nc.sync.dma_start(out=wx_t[:], in_=wxt[:])
        wy_t = sb.tile([H, S * 32], f16, tag="wy")

        # PE warmup: dense matmuls while the first feature DMAs land, to
        # flip the HAM clock gate to 8/8 (PE runs at 1.2 GHz until it sees
        # a ~3.4us window of sustained activity; everything after runs 2x)
        warm = sb.tile([128, 640], f16, tag="warm")
        nc.gpsimd.memset(warm[:], 0.0)
        # warmup matmuls use a pb-pool bank; quarter 0 has no stage-B
        # work, so this never blocks a copy
        wps = pb_pool.tile([128, 512], f32, tag="pb")
        for _ in range(10):
            nc.tensor.matmul(wps[:, 0:512], lhsT=warm[:, 0:128],
                             rhs=warm[:, 128:640])
        # per-quarter T1 tiles so stage-B reads of quarter j never create
        # false WAR deps against stage-A writes of quarter j+1
        t1 = [sb.tile([H, S, 64, POOLED], f16, tag=f"t1_{j}",
                      name=f"t1_{j}") for j in range(4)]

        chunk_idx = 0
        cur = None
        pa = None
        stgs = {}

        def keepalive(n=512):
            # real matmul: only MAC activity feeds the HAM activity
            # monitor (LDWEIGHTS/transpose don't count)
            ka = pb_pool.tile([128, 512], f32, tag="pb")
            nc.tensor.matmul(ka[:, 0:n], lhsT=warm[:, 0:128],
                             rhs=warm[:, 128:128 + n])

        def keepalive_pa():
            # tail-only keepalive: the pa pool is idle once stage A ends,
            # so these never steal a pb bank -- keepalives from pb during
            # the tail pin pb allocations to one alternating bank and
            # serialize each emit behind the previous emit's copy
            # (~950ns/emit instead of the copy-paced ~600ns)
            ka = pa_pool.tile([H, 4, 256], f32, tag="pa")
            nc.tensor.matmul(ka[:, 0, 0:256], lhsT=warm[:, 0:100],
                             rhs=warm[:, 128:384])

        def emit_b_group(j, g):
            # stage-B for quarter j, 4-ROI group g at PSUM partition
            # offsets 0/32/64/96 (concurrent col-group matmuls)
            hf, jj = divmod(j, 2)
            if jj == 0:
                stgs[g] = stg_pool.tile([128, 2, 448], f16, tag="stg",
                                        name=f"stg_{hf}_{g}")
            stg = stgs[g]
            g0 = 4 * g
            rois_g = list(range(g0, min(g0 + 4, S)))
            hi = 32 * len(rois_g)
            c0, c1 = 0, 64
            pb = pb_pool.tile([128, 512], f32, tag="pb")
            for i, s in enumerate(rois_g):
                nc.tensor.matmul(
                    pb[32 * i:32 * i + 32, 0:448],
                    lhsT=wy_t[:, s * 32:(s + 1) * 32],
                    rhs=t1[j][:, s, c0:c1, :],
                    tile_position=(0, 32 * i),
                )
            bal_copy(stg[0:hi, jj, :], pb[0:hi, 0:448], 448)
            if jj == 1:
                lo = min(hi + 8, 104)
                nc.gpsimd.dma_start(out=out[g, hf, 0:lo], in_=stg[0:lo])

        for j in range(4):
            # ---- Stage A quarter: T1_j[h, s, c', q] = F[c].T @ wx ----
            # with the previous quarter's stage-B groups interleaved every
            # 8 channels to fill the copy-paced gaps on the PE
            pending_b = list(range(NG)) if j > 0 else []
            for ci, c in enumerate(range(64 * j, 64 * (j + 1))):
                if c == starts[chunk_idx]:
                    sz = chunks[chunk_idx]
                    cur = fw_pool.tile([W, 16, H], f16, tag="fw")
                    src = fwt[c * W * H:(c + sz) * W * H].rearrange(
                        "(w cc h) -> w cc h", w=W, h=H)
                    nc.sync.dma_start(out=cur[:, 0:sz, :], in_=src)
                    chunk_idx += 1
                    coff = c
                    if c == 16:
                        # wy is first needed by stage-B of quarter 0,
                        # i.e. early in quarter 1; don't let it delay
                        # the first feature chunks
                        nc.sync.dma_start(out=wy_t[:], in_=wyt[:])
                if c % 4 == 0:
                    pa = pa_pool.tile([H, 4, 256], f32, tag="pa")
                nc.tensor.matmul(
                    pa[:, c % 4, 0:SQ],
                    lhsT=cur[:, c - coff, :],
                    rhs=wx_t[:],
                )
                if c % 4 == 3:
                    src = pa[:, :, 0:SQ].rearrange("h c (s q) -> h s c q",
                                                   q=POOLED)
                    dst = t1[j][:, :, (c - 3) % 64:(c - 3) % 64 + 4, :]
                    bal_copy(dst, src, 4 * SQ)
                if ci % 7 == 4 and pending_b:
                    emit_b_group(j - 1, pending_b.pop(0))
                elif j == 0 and (ci % 8 == 4 or (ci < 32 and ci % 4 == 1)):
                    # no stage-B work yet: keepalive matmul to hold the
                    # HAM clock gate at 8/8. Denser in the first half of
                    # quarter 0: the DMA-paced ramp leaves low-activity
                    # windows there, and a single core hitting a ~3.4us
                    # idle window re-throttles and drags the whole
                    # max-over-cores exec time
                    keepalive()
            for g in pending_b:
                emit_b_group(j - 1, g)
            # bridge keepalive over the quarter boundary (the next quarter's
            # first B group waits on this quarter's last copy; a PE hole
            # here trips the HAM re-throttle)
            keepalive()
        # extra bridge before the tail: B(3) group 0 waits the last A-copy
        for _ in range(2):
            keepalive_pa()
        # last quarter's stage B (dense tail) with keepalives
        for g in range(NG):
            emit_b_group(3, g)
            keepalive_pa()

    nc.compile()
    return nc


def _plan_shards(bi, n_rois):
    """Assign ROI indices to (wave, core, slot). Returns S and a list of
    per-wave assignment arrays of shape [N_CORES, S] (-1 = padding)."""
    groups = [np.where(bi == b)[0] for b in range(B)]
    need = max((len(g) + CORES_PER_BATCH - 1) // CORES_PER_BATCH
               for g in groups)
    need = max(need, 1)
    S = min(need, S_CAP)
    per_wave_cap = S * CORES_PER_BATCH
    n_waves = max(-(-len(g) // per_wave_cap) for g in groups)
    waves = []
    for wv in range(n_waves):
        asg = np.full((N_CORES, S), -1, dtype=np.int64)
        for b in range(B):
            g = groups[b][wv * per_wave_cap:(wv + 1) * per_wave_cap]
            for k in range(CORES_PER_BATCH):
                chunk = g[k * S:(k + 1) * S]
                asg[b * CORES_PER_BATCH + k, :len(chunk)] = chunk
        waves.append(asg)
    return S, waves


def kernel(features, rois, _trace=False):
    from concourse.bass_utils import run_bass_kernel_spmd

    features = np.asarray(features, dtype=np.float32)
    rois = np.asarray(rois, dtype=np.float32)
    n_rois = rois.shape[0]
    bi = np.rint(rois[:, 0]).astype(np.int64)
    bi = np.where((bi >= 0) & (bi < B), bi, -1)

    wy, wx = _host_weights(rois)  # [N, 7, H] / [N, 7, W], 1/area folded
    S, waves = _plan_shards(bi, n_rois)

    if S not in _prog_cache:
        _prog_cache[S] = _build_program(S)
    nc = _prog_cache[S]

    # Features per batch as a flat buffer of chunk-contiguous [W, sz, H]
    # blocks (fast contiguous DRAM reads). Shared across the 4 cores of
    # each batch group.
    fwt = []
    for b in range(B):
        t = features[b].transpose(2, 0, 1).astype(np.float16)  # [W, C, H]
        fwt.append(np.concatenate(
            [t[:, c0:c1, :].ravel()
             for c0, c1 in zip(_STARTS[:-1], _STARTS[1:])]))

    out_full = np.zeros((n_rois, C, POOLED, POOLED), dtype=np.float32)
    exec_ns = None
    for asg in waves:
        in_maps = []
        for k in range(N_CORES):
            wxt = np.zeros((W, S * POOLED), dtype=np.float16)
            wyt = np.zeros((H, S * 32), dtype=np.float16)
            for s in range(S):
                r = asg[k, s]
                if r < 0:
                    continue
                # w[n, q, w-axis] -> [w-axis, s*7+q]
                wxt[:, s * POOLED:(s + 1) * POOLED] = \
                    wx[r].T.astype(np.float16)
                wyt[:, s * 32:s * 32 + POOLED] = \
                    wy[r].T.astype(np.float16)
            in_maps.append({
                "fwt": fwt[k // CORES_PER_BATCH],
                "wxt": wxt,
                "wyt": wyt,
            })
        res = run_bass_kernel_spmd(nc, in_maps, list(range(N_CORES)),
                                   trace=_trace)
        if res.exec_time_ns is not None:
            exec_ns = max(exec_ns or 0, res.exec_time_ns)
        for k in range(N_CORES):
            arr = res.results[k]["out"]  # [NG, 2, 104, 2, 448] f16
            for s in range(S):
                r = asg[k, s]
                if r < 0:
                    continue
                g, i = divmod(s, 4)
                # [hf, p, jj, c', q] -> [hf, jj, c', p, q] -> [C, 7, 7]
                blk = (arr[g, :, 32 * i:32 * i + POOLED]
                       .reshape(2, POOLED, 2, 64, POOLED)
                       .transpose(0, 2, 3, 1, 4)
                       .reshape(C, POOLED, POOLED))
                out_full[r] = blk.astype(np.float32)

    if _trace:
        kernel.last_exec_time_ns = exec_ns
    return out_full


# revision 57
# speedup vs baseline: 1.0338x; 1.0012x over previous
"""PrRoIPool (Precise RoI Pooling) Trainium2 Bass kernel.

Problem: features [2, 256, 100, 100] f32, rois [256, 5] f32 ->
out [256, 256, 7, 7] f32 where
  out[n,c,p,q] = (1/area) * sum_{h,w} F[bi,c,h,w] * wy[n,p,h] * wx[n,q,w]
with wy/wx the exact integrals of the bilinear-interp hat functions over
each pooling bin (separable).

Strategy (8 NeuronCores, SPMD):
  - Host: compute hat-integral weights wy [N,7,H], wx [N,7,W] (tiny:
    ~0.004%% of total FLOPs), fold 1/bin_h into wy and 1/bin_w into wx.
  - Shard ROIs by batch image: cores 0-3 take batch-0 ROIs, cores 4-7
    batch-1 (S slots per core, zero-padded). Each core holds the full
    feature image of its batch, pre-transposed to [W, C, H] fp16.
  - Stage A (per channel c): T1[h, (s,q)] = F[c].T @ wx  -- PE matmul,
    stationary [w=100, h=100], moving [w=100, S*7], fp32 PSUM, then
    cast-copy PSUM->SBUF fp16 (Vector/Scalar engines, load-balanced).
  - Stage B (per 4-ROI group, per 64-channel quarter j):
    out[p, (c',q)] = wy_s.T @ T1[:, s, 64j:64j+64, :]  -- 4 ROIs packed
    at PSUM partition offsets 0/32/64/96 via matmul tile_position; the 4
    matmuls run concurrently in distinct array col-groups.
  - Output DMA ships only the 7 valid rows of each 32-row slot block.
  - Host reassembles [N, C, 7, 7].
"""

import sys

if "/opt/trn_rl_repo" not in sys.path:
    sys.path.insert(0, "/opt/trn_rl_repo")

import numpy as np

POOLED = 7
SPATIAL_SCALE = 0.0625
B, C, H, W = 2, 256, 100, 100
N_CORES = 8
CORES_PER_BATCH = 4
S_CAP = 36  # max ROI slots per core (stage-A PSUM slot is half a bank)

_prog_cache = {}

# feature-chunk schedule: small chunks first for a fast pipeline start.
# Chunks are channel-contiguous; the host packs the feature image into a
# flat DRAM buffer as consecutive [W, chunk, H] blocks so every chunk DMA
# reads fully contiguous DRAM (strided reads of [W, C, H] run at ~80 GB/s
# and starve stage A).
_CHUNKS = []
for _q in ([2, 2, 4, 8, 16, 16, 16], [16, 16, 16, 16],
           [16, 16, 16, 16], [16, 16, 16, 16]):
    _CHUNKS.extend(_q)
_STARTS = np.cumsum([0] + _CHUNKS).tolist()


def _hat_cdf(u):
    return np.where(
        u <= 0.0,
        0.5 * np.clip(u + 1.0, 0.0, 1.0) ** 2,
        1.0 - 0.5 * np.clip(1.0 - u, 0.0, 1.0) ** 2,
    )


def _bin_weights(lo, hi, size):
    # [N, P] bounds -> [N, P, size] integral of hat centered at each index
    idx = np.arange(size, dtype=lo.dtype)
    return _hat_cdf(hi[..., None] - idx) - _hat_cdf(lo[..., None] - idx)


def _host_weights(rois):
    """Per-ROI separable weights with 1/area folded in. float32."""
    r = rois.astype(np.float64)
    x1 = r[:, 1] * SPATIAL_SCALE
    y1 = r[:, 2] * SPATIAL_SCALE
    x2 = r[:, 3] * SPATIAL_SCALE
    y2 = r[:, 4] * SPATIAL_SCALE
    bw = (x2 - x1) / POOLED
    bh = (y2 - y1) / POOLED
    ph = np.arange(POOLED, dtype=np.float64)
    ylo = y1[:, None] + ph * bh[:, None]
    yhi = ylo + bh[:, None]
    xlo = x1[:, None] + ph * bw[:, None]
    xhi = xlo + bw[:, None]
    wy = _bin_weights(ylo, yhi, H)  # [N, 7, H]
    wx = _bin_weights(xlo, xhi, W)  # [N, 7, W]
    # reference: out = einsum / max(area,1e-12) where area = bw*bh, zeroed
    # if area <= 0. Fold 1/bh into wy, 1/bw into wx (area > 0 case).
    ok = (bw * bh) > 0.0
    inv_bh = np.where(ok, 1.0 / np.maximum(bh, 1e-12), 0.0)
    inv_bw = np.where(ok, 1.0 / np.maximum(bw, 1e-12), 0.0)
    wy = wy * inv_bh[:, None, None]
    wx = wx * inv_bw[:, None, None]
    return wy.astype(np.float32), wx.astype(np.float32)


def _build_program(S):
    """Bass/Tile SPMD program for S ROI slots per core. Cached per S."""
    from contextlib import ExitStack

    from concourse import bacc, mybir
    import concourse.tile as tile

    f16 = mybir.dt.float16
    f32 = mybir.dt.float32
    SQ = S * POOLED
    assert SQ <= 256  # stage-A psum: 2 channel slots of 256 f32 = 1 bank
    NG = -(-S // 4)  # 4-slot stage-B groups

    nc = bacc.Bacc("TRN2", target_bir_lowering=False, debug=False,
                   num_devices=N_CORES)
    fwt = nc.dram_tensor("fwt", [W * C * H], f16, kind="ExternalInput")
    wxt = nc.dram_tensor("wxt", [W, SQ], f16, kind="ExternalInput")
    # wyt is padded to 32 cols per slot (25 zero) so each stage-B matmul
    # writes a full 32-partition PSUM block (cost is N-driven, M is free).
    wyt = nc.dram_tensor("wyt", [H, S * 32], f16, kind="ExternalInput")
    # staged output, fp16: per (group, channel-half) rows 0-103 of the
    # 4x32-row slot blocks (rows 32i..32i+6 valid); one DMA each keeps
    # the descriptor-gen cost low, and the 104-row block size keeps
    # every (g, hf) DRAM block 2KB-aligned (103 rows shatters the DMA
    # into ~350B packets and the drain takes ~60us)
    out = nc.dram_tensor("out", [NG, 2, 104, 2, 448], f16,
                         kind="ExternalOutput")

    chunks, starts = _CHUNKS, _STARTS

    # strict V/S alternation: the two copy engines run concurrently on
    # different PSUM banks; any consecutive same-engine pair serializes
    # on that engine's DRAIN while the other idles
    state = {"n_copy": 0}

    with tile.TileContext(nc) as tc, ExitStack() as ctx:
        sb = ctx.enter_context(tc.tile_pool(name="sb", bufs=1))
        fw_pool = ctx.enter_context(tc.tile_pool(name="fw", bufs=7))
        pa_pool = ctx.enter_context(tc.tile_pool(name="pa", bufs=3,
                                                 space="PSUM"))
        pb_pool = ctx.enter_context(tc.tile_pool(name="pb", bufs=2,
                                                 space="PSUM"))
        stg_pool = ctx.enter_context(tc.tile_pool(name="stg", bufs=NG + 2))

        def bal_copy(dst, src, fd):
            # scheduler-assigned engine: Tile knows the dependency graph
            # and picks Vector or Scalar per copy
            nc.any.tensor_copy(dst, src)

        wx_t = sb.tile([W, SQ], f16, tag="wx")
        nc.sync.dma_start(out=wx_t[:], in_=wxt[:])
        wy_t = sb.tile([H, S * 32], f16, tag="wy")

        # PE warmup: dense matmuls while the first feature DMAs land, to
        # flip the HAM clock gate to 8/8 (PE runs at 1.2 GHz until it sees
        # a ~3.4us window of sustained activity; everything after runs 2x)
        warm = sb.tile([128, 640], f16, tag="warm")
        nc.gpsimd.memset(warm[:], 0.0)
        # warmup matmuls use a pb-pool bank; quarter 0 has no stage-B
        # work, so this never blocks a copy
        wps = pb_pool.tile([128, 512], f32, tag="pb")
        for _ in range(10):
            nc.tensor.matmul(wps[:, 0:512], lhsT=warm[:, 0:128],
                             rhs=warm[:, 128:640])
        # per-quarter T1 tiles so stage-B reads of quarter j never create
        # false WAR deps against stage-A writes of quarter j+1
        t1 = [sb.tile([H, S, 64, POOLED], f16, tag=f"t1_{j}",
                      name=f"t1_{j}") for j in range(4)]

        chunk_idx = 0
        cur = None
        pa = None
        stgs = {}

        def keepalive(n=512):
            # real matmul: only MAC activity feeds the HAM activity
            # monitor (LDWEIGHTS/transpose don't count)
            ka = pb_pool.tile([128, 512], f32, tag="pb")
            nc.tensor.matmul(ka[:, 0:n], lhsT=warm[:, 0:128],
                             rhs=warm[:, 128:128 + n])

        def keepalive_pa():
            # tail-only keepalive: the pa pool is idle once stage A ends,
            # so these never steal a pb bank -- keepalives from pb during
            # the tail pin pb allocations to one alternating bank and
            # serialize each emit behind the previous emit's copy
            # (~950ns/emit instead of the copy-paced ~600ns)
            ka = pa_pool.tile([H, 4, 256], f32, tag="pa")
            nc.tensor.matmul(ka[:, 0, 0:256], lhsT=warm[:, 0:100],
                             rhs=warm[:, 128:384])

        def emit_b_group(j, g):
            # stage-B for quarter j, 4-ROI group g at PSUM partition
            # offsets 0/32/64/96 (concurrent col-group matmuls)
            hf, jj = divmod(j, 2)
            if jj == 0:
                stgs[g] = stg_pool.tile([128, 2, 448], f16, tag="stg",
                                        name=f"stg_{hf}_{g}")
            stg = stgs[g]
            g0 = 4 * g
            rois_g = list(range(g0, min(g0 + 4, S)))
            hi = 32 * len(rois_g)
            c0, c1 = 0, 64
            pb = pb_pool.tile([128, 512], f32, tag="pb")
            for i, s in enumerate(rois_g):
                nc.tensor.matmul(
                    pb[32 * i:32 * i + 32, 0:448],
                    lhsT=wy_t[:, s * 32:(s + 1) * 32],
                    rhs=t1[j][:, s, c0:c1, :],
                    tile_position=(0, 32 * i),
                )
            bal_copy(stg[0:hi, jj, :], pb[0:hi, 0:448], 448)
            if jj == 1:
                lo = min(hi + 8, 104)
                nc.gpsimd.dma_start(out=out[g, hf, 0:lo], in_=stg[0:lo])

        for j in range(4):
            # ---- Stage A quarter: T1_j[h, s, c', q] = F[c].T @ wx ----
            # with the previous quarter's stage-B groups interleaved every
            # 8 channels to fill the copy-paced gaps on the PE
            pending_b = list(range(NG)) if j > 0 else []
            for ci, c in enumerate(range(64 * j, 64 * (j + 1))):
                if c == starts[chunk_idx]:
                    sz = chunks[chunk_idx]
                    cur = fw_pool.tile([W, 16, H], f16, tag="fw")
                    src = fwt[c * W * H:(c + sz) * W * H].rearrange(
                        "(w cc h) -> w cc h", w=W, h=H)
                    nc.sync.dma_start(out=cur[:, 0:sz, :], in_=src)
                    chunk_idx += 1
                    coff = c
                    if c == 16:
                        # wy is first needed by stage-B of quarter 0,
                        # i.e. early in quarter 1; don't let it delay
                        # the first feature chunks
                        nc.sync.dma_start(out=wy_t[:], in_=wyt[:])
                if c % 4 == 0:
                    pa = pa_pool.tile([H, 4, 256], f32, tag="pa")
                nc.tensor.matmul(
                    pa[:, c % 4, 0:SQ],
                    lhsT=cur[:, c - coff, :],
                    rhs=wx_t[:],
                )
                if c % 4 == 3:
                    src = pa[:, :, 0:SQ].rearrange("h c (s q) -> h s c q",
                                                   q=POOLED)
                    dst = t1[j][:, :, (c - 3) % 64:(c - 3) % 64 + 4, :]
                    bal_copy(dst, src, 4 * SQ)
                if ci % 7 == 4 and pending_b:
                    emit_b_group(j - 1, pending_b.pop(0))
                elif j == 0 and (ci % 8 == 4 or (ci < 32 and ci % 4 == 1)):
                    # no stage-B work yet: keepalive matmul to hold the
                    # HAM clock gate at 8/8. Denser in the first half of
                    # quarter 0: the DMA-paced ramp leaves low-activity
                    # windows there, and a single core hitting a ~3.4us
                    # idle window re-throttles and drags the whole
                    # max-over-cores exec time
                    keepalive()
            for g in pending_b:
                emit_b_group(j - 1, g)
            # bridge keepalive over the quarter boundary (the next quarter's
            # first B group waits on this quarter's last copy; a PE hole
            # here trips the HAM re-throttle)
            keepalive()
        # extra bridge before the tail: B(3) group 0 waits the last A-copy
        for _ in range(2):
            keepalive_pa()
        # last quarter's stage B (dense tail) with keepalives
        for g in range(NG):
            emit_b_group(3, g)
            keepalive_pa()

    nc.compile()
    return nc


def _plan_shards(bi, n_rois):
    """Assign ROI indices to (wave, core, slot). Returns S and a list of
    per-wave assignment arrays of shape [N_CORES, S] (-1 = padding)."""
    groups = [np.where(bi == b)[0] for b in range(B)]
    need = max((len(g) + CORES_PER_BATCH - 1) // CORES_PER_BATCH
               for g in groups)
    need = max(need, 1)
    S = min(need, S_CAP)
    per_wave_cap = S * CORES_PER_BATCH
    n_waves = max(-(-len(g) // per_wave_cap) for g in groups)
    waves = []
    for wv in range(n_waves):
        asg = np.full((N_CORES, S), -1, dtype=np.int64)
        for b in range(B):
            g = groups[b][wv * per_wave_cap:(wv + 1) * per_wave_cap]
            for k in range(CORES_PER_BATCH):
                chunk = g[k * S:(k + 1) * S]
                asg[b * CORES_PER_BATCH + k, :len(chunk)] = chunk
        waves.append(asg)
    return S, waves


def kernel(features, rois, _trace=False):
    from concourse.bass_utils import run_bass_kernel_spmd

    features = np.asarray(features, dtype=np.float32)
    rois = np.asarray(rois, dtype=np.float32)
    n_rois = rois.shape[0]
    bi = np.rint(rois[:, 0]).astype(np.int64)
    bi = np.where((bi >= 0) & (bi < B), bi, -1)

    wy, wx = _host_weights(rois)  # [N, 7, H] / [N, 7, W], 1/area folded
    S, waves = _plan_shards(bi, n_rois)

    if S not in _prog_cache:
        _prog_cache[S] = _build_program(S)
    nc = _prog_cache[S]

    # Features per batch as a flat buffer of chunk-contiguous [W, sz, H]
    # blocks (fast contiguous DRAM reads). Shared across the 4 cores of
    # each batch group.
    fwt = []
    for b in range(B):
        t = features[b].transpose(2, 0, 1).astype(np.float16)  # [W, C, H]
        fwt.append(np.concatenate(
            [t[:, c0:c1, :].ravel()
             for c0, c1 in zip(_STARTS[:-1], _STARTS[1:])]))

    out_full = np.zeros((n_rois, C, POOLED, POOLED), dtype=np.float32)
    exec_ns = None
    for asg in waves:
        in_maps = []
        for k in range(N_CORES):
            wxt = np.zeros((W, S * POOLED), dtype=np.float16)
            wyt = np.zeros((H, S * 32), dtype=np.float16)
            for s in range(S):
                r = asg[k, s]
                if r < 0:
                    continue
                # w[n, q, w-axis] -> [w-axis, s*7+q]
                wxt[:, s * POOLED:(s + 1) * POOLED] = \
                    wx[r].T.astype(np.float16)
                wyt[:, s * 32:s * 32 + POOLED] = \
                    wy[r].T.astype(np.float16)
            in_maps.append({
                "fwt": fwt[k // CORES_PER_BATCH],
                "wxt": wxt,
                "wyt": wyt,
            })
        res = run_bass_kernel_spmd(nc, in_maps, list(range(N_CORES)),
                                   trace=_trace)
        if res.exec_time_ns is not None:
            exec_ns = max(exec_ns or 0, res.exec_time_ns)
        for k in range(N_CORES):
            arr = res.results[k]["out"]  # [NG, 2, 104, 2, 448] f16
            for s in range(S):
                r = asg[k, s]
                if r < 0:
                    continue
                g, i = divmod(s, 4)
                # [hf, p, jj, c', q] -> [hf, jj, c', p, q] -> [C, 7, 7]
                blk = (arr[g, :, 32 * i:32 * i + POOLED]
                       .reshape(2, POOLED, 2, 64, POOLED)
                       .transpose(0, 2, 3, 1, 4)
                       .reshape(C, POOLED, POOLED))
                out_full[r] = blk.astype(np.float32)

    if _trace:
        kernel.last_exec_time_ns = exec_ns
    return out_full
